# revision 1
# baseline (speedup 1.0000x reference)
"""BitLinear (RMSNorm + int8-absmax activation quant + ternary weight quant + matmul)
on 8 Trainium2 NeuronCores.

Strategy:
  - Shard rows of x across cores (256 rows each): RMSNorm + local absmax.
  - Shard weight columns across cores ([4096, 512] each): local sum(|W|).
  - AllGather the two scalars -> global a_scale / b_scale (exact semantics).
  - Quantize activations to bf16 ints in [-127, 127] (exact in bf16),
    AllGather the quantized activation matrix (bf16, 16.8 MB total).
  - Quantize local weight shard to ternary bf16.
  - Matmul A_q @ B_t per core: lhsT tiles come from hardware DMA-transpose
    loads of the gathered bf16 activations; accumulate K=4096 in PSUM over
    32 k-tiles; dequant fused into the PSUM->SBUF copy.
  - Each core writes its [2048, 512] output column shard; host concatenates.

Self-contained: only needs numpy + the platform's concourse/bass libraries.
"""

import os
import sys

import numpy as np

for _p in ("/opt/trn_rl_repo", "/root/.axon_site/_ro/trn_rl_repo"):
    if os.path.isdir(_p) and _p not in sys.path:
        sys.path.append(_p)

import concourse.bass as bass
import concourse.tile as tile
from concourse import mybir
from concourse.bass_utils import run_bass_kernel_spmd

R = 8  # cores
M, K, N = 2048, 4096, 4096
M_LOC = M // R  # 256 rows of x per core
N_LOC = N // R  # 512 weight columns per core
P = 128
KT = K // P  # 32 k-tiles
MT_LOC = M_LOC // P  # 2 m-tiles per core
EPS_RMS = 1e-6
Q_CLIP = 1e-5
MAGIC = 12582912.0  # 1.5 * 2**23: (v + MAGIC) - MAGIC == round-to-nearest-even(v)
F32 = mybir.dt.float32
BF16 = mybir.dt.bfloat16
AX = mybir.AxisListType
ALU = mybir.AluOpType


def _split_waits(nc, max_waits=1):
    """This toolchain rejects instructions with several semaphore waits
    ("Too many sync wait commands"). Hoist excess waits onto no-op
    instructions just before the offender on the same engine."""
    counter = 0
    for f in nc.m.functions:
        for blk in f.blocks:
            new_insts = []
            for inst in blk.instructions:
                si = getattr(inst, "sync_info", None)
                waits = list(si.on_wait) if si is not None and si.on_wait else []
                if len(waits) > max_waits:
                    excess = waits[: len(waits) - max_waits]
                    keep = waits[len(waits) - max_waits :]
                    for i in range(0, len(excess), max_waits):
                        counter += 1
                        nop = mybir.InstNoOp(
                            name=f"waitsplit_{counter}_{inst.name}", ins=[], outs=[]
                        )
                        nop.engine = inst.engine
                        nop.bass_nofuse = True
                        nop.sync_info = mybir.SyncInfo(
                            on_wait=list(excess[i : i + max_waits]), on_update=[]
                        )
                        new_insts.append(nop)
                    si.on_wait = keep
                    inst.sync_info = si
                new_insts.append(inst)
            blk.instructions[:] = new_insts


def _bcast_ap(ap, p):
    """Broadcast a 1-D DRAM AP across p partitions (step-0 partition axis)."""
    return bass.AP(tensor=ap.tensor, offset=ap.offset, ap=[[0, p]] + list(ap.ap))


def build_kernel(reps=1, stop_after=None, mode=None):
    nc = bass.Bass(num_devices=R)
    rg = [list(range(R))]

    x_in = nc.declare_dram_parameter("x_loc", [M_LOC, K], F32, isOutput=False)
    w_in = nc.declare_dram_parameter("w_loc", [K, N_LOC], F32, isOutput=False)
    rms_in = nc.declare_dram_parameter("rms_w", [K], F32, isOutput=False)
    out_ext = nc.declare_dram_parameter("out_loc", [M, N_LOC], F32, isOutput=True)

    stats_loc = nc.dram_tensor("stats_loc", [P * 2], F32)
    stats_all = nc.dram_tensor("stats_all", [R * P * 2], F32, addr_space="Shared")
    wstat_loc = nc.dram_tensor("wstat_loc", [P], F32)
    wstat_all = nc.dram_tensor("wstat_all", [R * P], F32, addr_space="Shared")
    scal_dram = nc.dram_tensor("scal_dram", [1], F32)
    scbd_dram = nc.dram_tensor("scbd_dram", [2], F32)
    aq_loc = nc.dram_tensor("aq_loc", [M_LOC, K], BF16)
    aq_all_a = nc.dram_tensor("aq_all_a", [M // 2, K], BF16, addr_space="Shared")
    aq_all_b = nc.dram_tensor("aq_all_b", [M // 2, K], BF16, addr_space="Shared")

    with tile.TileContext(nc) as tc:
        ctxs = [
            tc.tile_pool(name="wres", bufs=1),
            tc.tile_pool(name="btres", bufs=1),
            tc.tile_pool(name="rmsp", bufs=1),
            tc.tile_pool(name="xz", bufs=2),
            tc.tile_pool(name="aq", bufs=2),
            tc.tile_pool(name="st", bufs=2),
            tc.tile_pool(name="lhs", bufs=12),
            tc.tile_pool(name="psum", bufs=8, space="PSUM"),
            tc.tile_pool(name="outp", bufs=4),
            tc.tile_pool(name="small", bufs=1),
        ]
        from contextlib import ExitStack

        with ExitStack() as es:
            (wres_p, bt_p, rms_p, xz_p, aq_p, st_p, lhs_p, psum_p, out_p, small_p) = [
                es.enter_context(c) for c in ctxs
            ]

            eps_t = small_p.tile([P, 1], F32)
            nc.vector.memset(eps_t, EPS_RMS)
            rms_b = rms_p.tile([P, K], F32)
            nc.scalar.dma_start(rms_b[:], _bcast_ap(rms_in[:], P))

            prep_state = None
            for _rep in range(reps):
                if mode == "mm_loop" and prep_state is not None:
                    emit_matmul(nc, aq_all_a, aq_all_b, out_ext,
                                prep_state[0], prep_state[1],
                                lhs_p, psum_p, out_p, _rep)
                    continue
                st = emit_body(nc, tc, rg, x_in, w_in, rms_in, out_ext,
                               stats_loc, stats_all, wstat_loc, wstat_all,
                               scal_dram, scbd_dram, aq_loc, aq_all_a, aq_all_b,
                               wres_p, bt_p, rms_p, xz_p, aq_p, st_p, lhs_p,
                               psum_p, out_p, small_p, eps_t, _rep, stop_after,
                               rms_b)
                if st is not None:
                    emit_matmul(nc, aq_all_a, aq_all_b, out_ext, st[0], st[1],
                                lhs_p, psum_p, out_p, _rep)
                    prep_state = st

    _split_waits(nc)
    return nc


def emit_body(nc, tc, rg, x_in, w_in, rms_in, out_ext,
              stats_loc, stats_all, wstat_loc, wstat_all,
              scal_dram, scbd_dram, aq_loc, aq_all_a, aq_all_b,
              wres_p, bt_p, rms_p, xz_p, aq_p, st_p, lhs_p,
              psum_p, out_p, small_p, eps_t, rep, stop_after=None, rms_b=None):
    if True:
        if True:

            # ---------- W shard: load resident + abs-sum stats ----------
            w_res = wres_p.tile([P, KT, N_LOC], F32)
            nc.scalar.dma_start(
                w_res[:], w_in[:, :].rearrange("(kt p) n -> p kt n", p=P)
            )
            wsum32 = small_p.tile([P, KT], F32)
            nc.vector.tensor_reduce(
                out=wsum32,
                in_=w_res[:],
                axis=AX.X,
                op=ALU.add,
                apply_absolute_value=True,
            )
            pp2 = small_p.tile([P, 2], F32)
            nc.vector.tensor_reduce(out=pp2[:, 1:2], in_=wsum32, axis=AX.X, op=ALU.add)

            # ---------- x rows: RMS norm + local absmax ----------
            amax_mt = small_p.tile([P, MT_LOC], F32)
            z_tiles = []
            r_tiles = []
            for mt in range(MT_LOC):
                xz = xz_p.tile([P, K], F32)
                nc.scalar.dma_start(xz[:], x_in[mt * P : (mt + 1) * P, :])
                xg = xz[:].rearrange("p (g d) -> p g d", d=512)
                stats6 = st_p.tile([P, 8, 6], F32)
                for g in range(8):
                    nc.vector.bn_stats(out=stats6[:, g, :], in_=xg[:, g, :])
                mv = st_p.tile([P, 2], F32)
                nc.vector.bn_aggr(out=mv, in_=stats6[:])
                # mean(x^2) = var + mean^2
                msq = st_p.tile([P, 1], F32)
                nc.vector.tensor_tensor(
                    out=msq, in0=mv[:, 0:1], in1=mv[:, 0:1], op=ALU.mult
                )
                nc.vector.tensor_tensor(out=msq, in0=msq, in1=mv[:, 1:2], op=ALU.add)
                # r = 1/sqrt(msq + eps)
                r_t = st_p.tile([P, 1], F32)
                nc.scalar.activation(
                    out=r_t,
                    in_=msq,
                    func=mybir.ActivationFunctionType.Sqrt,
                    bias=eps_t,
                    scale=1.0,
                )
                nc.vector.reciprocal(out=r_t, in_=r_t)
                # zz = x * rms_weight ; per-row absmax of zz
                amax_raw = st_p.tile([P, 1], F32)
                nc.vector.tensor_tensor(
                    out=xz[:], in0=xz[:], in1=rms_b[:], op=ALU.mult
                )
                nc.vector.tensor_reduce(
                    out=amax_raw,
                    in_=xz[:],
                    axis=AX.X,
                    op=ALU.max,
                    apply_absolute_value=True,
                )
                # row absmax of z = absmax(x*rms) * r  (r > 0)
                nc.vector.tensor_tensor(
                    out=amax_mt[:, mt : mt + 1], in0=amax_raw, in1=r_t, op=ALU.mult
                )
                r_tiles.append(r_t)
                z_tiles.append(xz)

            nc.vector.tensor_reduce(
                out=pp2[:, 0:1], in_=amax_mt[:], axis=AX.X, op=ALU.max
            )

            # ---------- AG1: merged stats partials ([P,2] per rank) ----------
            nc.scalar.dma_start(stats_loc[:].rearrange("(p t) -> p t", p=P), pp2[:])
            nc.gpsimd.collective_compute(
                "AllGather",
                ALU.bypass,
                replica_groups=rg,
                ins=[stats_loc[:]],
                outs=[stats_all[:]],
            )
            sball = small_p.tile([P, R * P * 2], F32)
            nc.scalar.dma_start(sball[:], _bcast_ap(stats_all[:], P))
            v = sball[:].rearrange("p (r t) -> p r t", t=2)
            gmax = small_p.tile([P, 1], F32)
            nc.vector.tensor_reduce(out=gmax, in_=v[:, :, 0:1], axis=AX.XY, op=ALU.max)
            nc.vector.tensor_scalar_max(out=gmax, in0=gmax, scalar1=Q_CLIP)
            a_s = small_p.tile([P, 1], F32)
            nc.vector.reciprocal(out=a_s, in_=gmax)
            nc.vector.tensor_scalar_mul(out=a_s, in0=a_s, scalar1=127.0)

            gsum = small_p.tile([P, 1], F32)
            nc.vector.tensor_reduce(out=gsum, in_=v[:, :, 1:2], axis=AX.XY, op=ALU.add)
            nc.vector.tensor_scalar(
                out=gsum,
                in0=gsum,
                scalar1=1.0 / (K * N),
                scalar2=Q_CLIP,
                op0=ALU.mult,
                op1=ALU.max,
            )
            b_s = small_p.tile([P, 1], F32)
            nc.vector.reciprocal(out=b_s, in_=gsum)
            dq_b = small_p.tile([P, 1], F32)
            nc.vector.tensor_tensor(out=dq_b, in0=gmax, in1=gsum, op=ALU.mult)
            nc.vector.tensor_scalar_mul(out=dq_b, in0=dq_b, scalar1=1.0 / 127.0)

            if stop_after == "scales":
                nc.scalar.dma_start(out_ext[0:P, 0:1], dq_b[:])
                return

            # ---------- activation quant (bf16 ints) + split allgather -------
            for mt in range(MT_LOC):
                z = z_tiles[mt]
                rs_c = st_p.tile([P, 1], F32, tag="rs_c", name=f"rs_{mt}")
                nc.vector.tensor_tensor(
                    out=rs_c, in0=r_tiles[mt], in1=a_s, op=ALU.mult
                )
                # z <- z * (r*a_scale) + MAGIC ; aq <- z - MAGIC (round nearest)
                nc.vector.tensor_scalar(
                    out=z[:],
                    in0=z[:],
                    scalar1=rs_c,
                    scalar2=MAGIC,
                    op0=ALU.mult,
                    op1=ALU.add,
                )
                aq_t = aq_p.tile([P, K], BF16)
                nc.vector.tensor_scalar(
                    out=aq_t[:], in0=z[:], scalar1=MAGIC, scalar2=None, op0=ALU.subtract
                )
                nc.scalar.dma_start(aq_loc[mt * P : (mt + 1) * P, :], aq_t[:])
                nc.gpsimd.collective_compute(
                    "AllGather",
                    ALU.bypass,
                    replica_groups=rg,
                    ins=[aq_loc[mt * P : (mt + 1) * P, :]],
                    outs=[(aq_all_a if mt == 0 else aq_all_b)[:, :]],
                )

            # ---------- weight quant: ternary bf16 ----------
            bt = bt_p.tile([P, KT, N_LOC], BF16)
            nc.vector.tensor_scalar(
                out=w_res[:],
                in0=w_res[:],
                scalar1=b_s[:, 0:1],
                scalar2=MAGIC,
                op0=ALU.mult,
                op1=ALU.add,
            )
            nc.vector.tensor_scalar(
                out=w_res[:],
                in0=w_res[:],
                scalar1=MAGIC,
                scalar2=1.0,
                op0=ALU.subtract,
                op1=ALU.min,
            )
            nc.vector.tensor_scalar(
                out=bt[:], in0=w_res[:], scalar1=-1.0, scalar2=None, op0=ALU.max
            )

            if stop_after == "quant":
                o_t = out_p.tile([P, N_LOC], F32)
                nc.vector.tensor_scalar_mul(out=o_t[:], in0=bt[:, 0, :], scalar1=1.0)
                nc.scalar.dma_start(out_ext[0:P, :], o_t[:])
                return None
            return (bt, dq_b)


def emit_matmul(nc, aq_all_a, aq_all_b, out_ext, bt, scal_b,
                lhs_p, psum_p, out_p, rep):
    if True:
        if True:
            # ---------- matmul: out[m, n_loc] = A_q @ B_t, dequant fused -----
            # half 0 consumes aq_all_a (each rank's first m-tile: global
            # m-tiles 0,2,4,...), half 1 consumes aq_all_b (1,3,5,...), so the
            # second allgather overlaps the first half's matmuls.
            HALF_MT = 8
            for half in range(2):
                psums = [
                    psum_p.tile([P, N_LOC], F32, tag="ps", name=f"ps_{half}_{i}")
                    for i in range(HALF_MT)
                ]
                aq_src = aq_all_a if half == 0 else aq_all_b
                for kt in range(KT):
                    lhsT = lhs_p.tile(
                        [P, HALF_MT * P], BF16, tag="lhsT", name=f"lh_{half}_{kt}"
                    )
                    nc.sync.dma_start_transpose(
                        lhsT[:], aq_src[:, kt * P : (kt + 1) * P]
                    )
                    for mt in range(HALF_MT):
                        nc.tensor.matmul(
                            psums[mt][:],
                            lhsT[:, mt * P : (mt + 1) * P],
                            bt[:, kt, :],
                            start=(kt == 0),
                            stop=(kt == KT - 1),
                        )
                for mt in range(HALF_MT):
                    o_t = out_p.tile([P, N_LOC], F32)
                    nc.vector.tensor_scalar_mul(
                        out=o_t[:], in0=psums[mt][:], scalar1=scal_b[:, 0:1]
                    )
                    gm = 2 * mt + half
                    nc.scalar.dma_start(out_ext[gm * P : (gm + 1) * P, :], o_t[:])


_CACHE = {}


def _get_nc():
    if "nc" not in _CACHE:
        _CACHE["nc"] = build_kernel()
    return _CACHE["nc"]


def make_in_maps(x, weight, rms_weight):
    x = np.ascontiguousarray(np.asarray(x, dtype=np.float32)).reshape(M, K)
    weight = np.asarray(weight, dtype=np.float32)
    rms_weight = np.ascontiguousarray(np.asarray(rms_weight, dtype=np.float32))
    return [
        {
            "x_loc": np.ascontiguousarray(x[c * M_LOC : (c + 1) * M_LOC]),
            "w_loc": np.ascontiguousarray(weight[:, c * N_LOC : (c + 1) * N_LOC]),
            "rms_w": rms_weight,
        }
        for c in range(R)
    ]


def assemble_out(results):
    out = np.concatenate([results[c]["out_loc"] for c in range(R)], axis=1)
    return out.reshape(1, M, N)


def kernel(x, weight, rms_weight):
    nc = _get_nc()
    in_maps = make_in_maps(x, weight, rms_weight)
    res = run_bass_kernel_spmd(nc, in_maps, core_ids=list(range(R)))
    return assemble_out(res.results)



# revision 2
# speedup vs baseline: 23.2148x; 23.2148x over previous
"""BitLinear (RMSNorm + int8-absmax activation quant + ternary weight quant
+ matmul) on 8 Trainium2 NeuronCores — v4.

Bit-faithful quantization (same numerics as the reference: exact int8
activation values, exact global a_scale/b_scale), restructured for
throughput:

  - x rows sharded (256/core): bn_stats row moments -> r, x*rms multiply,
    per-row absmax; weight columns sharded ([4096, 512]/core, loaded split
    across both DMA queues): local |W| sum. Both per-core scalars are
    partition-reduced on-chip via a tiny DRAM broadcast round-trip.
  - AllGather #1 (tiny, f32, exact): per-core (max|xn|, sum|W|) ->
    global a_scale / b_scale on every core.
  - Quantize activations to int8 values held in bf16 (exact), then ONE
    local DMA-transpose per m-tile into A^T layout [p, kt, m]
    (k = kt*128 + p), staged to DRAM.
  - AllGather #2/#3: the two transposed m-tiles; #3 overlaps the first
    matmul half.
  - Matmul: lhsT tiles are contiguous strided loads (alternating DMA
    queues) from the gathered transposed activations; 32 k-tiles
    accumulate in PSUM; dequant 1/(a_s*b_s) fused into the PSUM drain.
  - Ternarization of the weight shard is JIT-chunked after b_scale lands.
  - Each core writes its [2048, 512] output column shard; host
    concatenates.

In steady state every phase hides under the ~110us tensor-engine matmul
stream (512 bf16 [128,128]@[128,512] matmuls per core).

Self-contained: only needs numpy + the platform's concourse/bass libraries.
"""

import os
import sys

import numpy as np

for _p in ("/opt/trn_rl_repo", "/root/.axon_site/_ro/trn_rl_repo"):
    if os.path.isdir(_p) and _p not in sys.path:
        sys.path.append(_p)

import concourse.bass as bass
import concourse.tile as tile
from concourse import mybir
from concourse.bass_utils import run_bass_kernel_spmd

R = 8  # cores
M, K, N = 2048, 4096, 4096
M_LOC = M // R  # 256 rows of x per core
N_LOC = N // R  # 512 weight columns per core
P = 128
KT = K // P  # 32 k-tiles
MT_LOC = M_LOC // P  # 2 m-tiles per core
EPS_RMS = 1e-6
Q_CLIP = 1e-5
MAGIC = 12582912.0  # 1.5 * 2**23: (v + MAGIC) - MAGIC == round-to-nearest-even(v)
F32 = mybir.dt.float32
BF16 = mybir.dt.bfloat16
AX = mybir.AxisListType
ALU = mybir.AluOpType

ZT_ELEMS = KT * P * P  # 524288 bf16: one m-tile's transposed activations


def _split_waits(nc, max_waits=1):
    """This toolchain rejects instructions with several semaphore waits
    ("Too many sync wait commands"). Hoist excess waits onto no-op
    instructions just before the offender on the same engine."""
    counter = 0
    for f in nc.m.functions:
        for blk in f.blocks:
            new_insts = []
            for inst in blk.instructions:
                si = getattr(inst, "sync_info", None)
                waits = list(si.on_wait) if si is not None and si.on_wait else []
                if len(waits) > max_waits:
                    excess = waits[: len(waits) - max_waits]
                    keep = waits[len(waits) - max_waits :]
                    for i in range(0, len(excess), max_waits):
                        counter += 1
                        nop = mybir.InstNoOp(
                            name=f"waitsplit_{counter}_{inst.name}", ins=[], outs=[]
                        )
                        nop.engine = inst.engine
                        nop.bass_nofuse = True
                        nop.sync_info = mybir.SyncInfo(
                            on_wait=list(excess[i : i + max_waits]), on_update=[]
                        )
                        new_insts.append(nop)
                    si.on_wait = keep
                    inst.sync_info = si
                new_insts.append(inst)
            blk.instructions[:] = new_insts


def _bcast_ap(ap, p):
    """Broadcast a 1-D DRAM AP across p partitions (step-0 partition axis)."""
    return bass.AP(tensor=ap.tensor, offset=ap.offset, ap=[[0, p]] + list(ap.ap))


def build_kernel(reps=1, mode=None):
    nc = bass.Bass(num_devices=R)
    rg = [list(range(R))]

    x_in = nc.declare_dram_parameter("x_loc", [M_LOC, K], F32, isOutput=False)
    w_in = nc.declare_dram_parameter("w_loc", [K, N_LOC], F32, isOutput=False)
    rms_in = nc.declare_dram_parameter("rms_w", [K], F32, isOutput=False)
    out_ext = nc.declare_dram_parameter("out_loc", [M, N_LOC], F32, isOutput=True)

    sb_loc = nc.dram_tensor("sb_loc", [P * 2], F32)
    sb_all = nc.dram_tensor("sb_all", [R * P * 2], F32, addr_space="Shared")
    zt_loc_a = nc.dram_tensor("zt_loc_a", [ZT_ELEMS], BF16)
    zt_all_a = nc.dram_tensor("zt_all_a", [R * ZT_ELEMS], BF16, addr_space="Shared")
    zt_loc_b = nc.dram_tensor("zt_loc_b", [ZT_ELEMS], BF16)
    zt_all_b = nc.dram_tensor("zt_all_b", [R * ZT_ELEMS], BF16, addr_space="Shared")
    wsc_d = nc.dram_tensor("wsc_d", [P * 2], F32)

    with tile.TileContext(nc) as tc:
        ctxs = [
            tc.tile_pool(name="wres", bufs=1),
            tc.tile_pool(name="btres", bufs=1),
            tc.tile_pool(name="rmsp", bufs=1),
            tc.tile_pool(name="xz", bufs=2),
            tc.tile_pool(name="zb", bufs=2),
            tc.tile_pool(name="ztp", bufs=2),
            tc.tile_pool(name="st", bufs=2),
            tc.tile_pool(name="lhs", bufs=4),
            tc.tile_pool(name="psum", bufs=8, space="PSUM"),
            tc.tile_pool(name="outp", bufs=4),
            tc.tile_pool(name="small", bufs=1),
        ]
        from contextlib import ExitStack

        with ExitStack() as es:
            pools = [es.enter_context(c) for c in ctxs]
            (wres_p, bt_p, rms_p, xz_p, zb_p, ztp_p, st_p, lhs_p, psum_p,
             out_p, small_p) = pools

            eps_t = small_p.tile([P, 1], F32)
            nc.vector.memset(eps_t, EPS_RMS)

            prep_state = None
            for _rep in range(reps):
                if mode == "mm_loop" and prep_state is not None:
                    emit_matmul(nc, zt_all_a, zt_all_b,
                                out_ext, prep_state[0], prep_state[1],
                                lhs_p, psum_p, out_p, _rep)
                    continue
                st = emit_body(nc, rg, x_in, w_in, rms_in,
                               sb_loc, sb_all, zt_loc_a, zt_all_a,
                               zt_loc_b, zt_all_b, wsc_d,
                               wres_p, bt_p, rms_p, xz_p, zb_p, ztp_p, st_p,
                               small_p, eps_t, _rep)
                emit_matmul(nc, zt_all_a, zt_all_b, out_ext, st[0], st[1],
                            lhs_p, psum_p, out_p, _rep)
                prep_state = st

    _split_waits(nc)
    return nc


def emit_body(nc, rg, x_in, w_in, rms_in,
              sb_loc, sb_all, zt_loc_a, zt_all_a, zt_loc_b, zt_all_b, wsc_d,
              wres_p, bt_p, rms_p, xz_p, zb_p, ztp_p, st_p,
              small_p, eps_t, rep):
    # ---------- loads: W split across both DMA queues; x + rms on scalar ----
    w_res = wres_p.tile([P, KT, N_LOC], F32)
    HK = KT // 2
    nc.sync.dma_start(
        w_res[:, 0:HK, :],
        w_in[0 : HK * P, :].rearrange("(kt p) n -> p kt n", p=P),
    )
    xf_tiles = []
    for mt in range(MT_LOC):
        xf = xz_p.tile([P, K], F32)
        nc.scalar.dma_start(xf[:], x_in[mt * P : (mt + 1) * P, :])
        xf_tiles.append(xf)
    rms_b = rms_p.tile([P, K], F32)
    nc.scalar.dma_start(rms_b[:], _bcast_ap(rms_in[:], P))
    nc.scalar.dma_start(
        w_res[:, HK:KT, :],
        w_in[HK * P : K, :].rearrange("(kt p) n -> p kt n", p=P),
    )

    # ---------- per m-tile: moments -> r, x*rms, per-row absmax ------------
    amax_mt = small_p.tile([P, MT_LOC], F32, tag="amx", name=f"amx_{rep}")
    r_tiles = []
    for mt in range(MT_LOC):
        xf = xf_tiles[mt]
        xg = xf[:].rearrange("p (g d) -> p g d", d=512)
        stats6 = st_p.tile([P, 8, 6], F32)
        for g in range(8):
            nc.vector.bn_stats(out=stats6[:, g, :], in_=xg[:, g, :])
        mv = st_p.tile([P, 2], F32)
        nc.vector.bn_aggr(out=mv, in_=stats6[:])
        msq = st_p.tile([P, 1], F32, tag="msq", name=f"msq_{rep}_{mt}")
        nc.vector.tensor_tensor(out=msq, in0=mv[:, 0:1], in1=mv[:, 0:1],
                                op=ALU.mult)
        nc.vector.tensor_tensor(out=msq, in0=msq, in1=mv[:, 1:2], op=ALU.add)
        r_t = st_p.tile([P, 1], F32, tag="rt", name=f"rt_{rep}_{mt}")
        nc.scalar.activation(
            out=r_t, in_=msq, func=mybir.ActivationFunctionType.Sqrt,
            bias=eps_t, scale=1.0,
        )
        nc.vector.reciprocal(out=r_t, in_=r_t)
        r_tiles.append(r_t)
        # xf <- x * rms_w (in place); per-row absmax of xn = absmax(xf) * r
        nc.vector.tensor_tensor(out=xf[:], in0=xf[:], in1=rms_b[:], op=ALU.mult)
        amax_raw = st_p.tile([P, 1], F32, tag="amr", name=f"amr_{rep}_{mt}")
        nc.vector.tensor_reduce(
            out=amax_raw, in_=xf[:], axis=AX.X, op=ALU.max,
            apply_absolute_value=True,
        )
        nc.vector.tensor_tensor(
            out=amax_mt[:, mt : mt + 1], in0=amax_raw, in1=r_t, op=ALU.mult
        )

    # ---------- per-core scalars: partition-reduce via DRAM round-trip ------
    wsum32 = small_p.tile([P, KT], F32, tag="ws32", name=f"ws32_{rep}")
    nc.vector.tensor_reduce(
        out=wsum32[:, 0:HK], in_=w_res[:, 0:HK, :], axis=AX.X, op=ALU.add,
        apply_absolute_value=True,
    )
    nc.vector.tensor_reduce(
        out=wsum32[:, HK:KT], in_=w_res[:, HK:KT, :], axis=AX.X, op=ALU.add,
        apply_absolute_value=True,
    )
    pr = small_p.tile([P, 2], F32, tag="pr", name=f"pr_{rep}")
    nc.vector.tensor_reduce(out=pr[:, 0:1], in_=amax_mt[:], axis=AX.X, op=ALU.max)
    nc.vector.tensor_reduce(out=pr[:, 1:2], in_=wsum32, axis=AX.X, op=ALU.add)
    nc.sync.dma_start(wsc_d[:].rearrange("(p t) -> p t", p=P), pr[:])
    wscb = small_p.tile([P, P, 2], F32, tag="wscb", name=f"wscb_{rep}")
    nc.sync.dma_start(
        wscb[:],
        bass.AP(tensor=wsc_d[:].tensor, offset=0, ap=[[0, P], [2, P], [1, 2]]),
    )
    pc = small_p.tile([P, 2], F32, tag="pc", name=f"pc_{rep}")
    nc.vector.tensor_reduce(
        out=pc[:, 0:1], in_=wscb[:, :, 0:1], axis=AX.XY, op=ALU.max
    )
    nc.vector.tensor_reduce(
        out=pc[:, 1:2], in_=wscb[:, :, 1:2], axis=AX.XY, op=ALU.add
    )
    nc.sync.dma_start(sb_loc[:].rearrange("(p t) -> p t", p=P), pc[:])

    # ---------- AllGather #1: tiny exact f32 stats --------------------------
    nc.gpsimd.collective_compute(
        "AllGather", ALU.bypass, replica_groups=rg,
        ins=[sb_loc[:]], outs=[sb_all[:]],
    )

    # global a_scale / b_scale (identical on every core)
    stp = small_p.tile([P, R, 2], F32, tag="stp", name=f"stp_{rep}")
    nc.scalar.dma_start(
        stp[:],
        bass.AP(tensor=sb_all[:].tensor, offset=0,
                ap=[[2, P], [P * 2, R], [1, 2]]),
    )
    gmax = small_p.tile([P, 1], F32, tag="gmax", name=f"gmax_{rep}")
    nc.vector.tensor_reduce(out=gmax, in_=stp[:, :, 0:1], axis=AX.XY, op=ALU.max)
    nc.vector.tensor_scalar_max(out=gmax, in0=gmax, scalar1=Q_CLIP)
    a_s = small_p.tile([P, 1], F32, tag="as", name=f"as_{rep}")
    nc.vector.reciprocal(out=a_s, in_=gmax)
    nc.vector.tensor_scalar_mul(out=a_s, in0=a_s, scalar1=127.0)
    gsum = small_p.tile([P, 1], F32, tag="gsum", name=f"gsum_{rep}")
    nc.vector.tensor_reduce(out=gsum, in_=stp[:, :, 1:2], axis=AX.XY, op=ALU.add)
    # gsum <- clip(mean|W|) = max(gsum/(K*N), Q_CLIP) == 1/b_scale
    nc.vector.tensor_scalar(
        out=gsum, in0=gsum, scalar1=1.0 / (K * N), scalar2=Q_CLIP,
        op0=ALU.mult, op1=ALU.max,
    )
    b_s = small_p.tile([P, 1], F32, tag="bs", name=f"bs_{rep}")
    nc.vector.reciprocal(out=b_s, in_=gsum)
    # dequant: 1/(a_s*b_s) = gmax_clipped * gsum_clipped / 127
    dq = small_p.tile([P, 1], F32, tag="dq", name=f"dq_{rep}")
    nc.vector.tensor_tensor(out=dq, in0=gmax, in1=gsum, op=ALU.mult)
    nc.vector.tensor_scalar_mul(out=dq, in0=dq, scalar1=1.0 / 127.0)

    # ---------- quantize + transpose + stage + AllGather #2/#3 -------------
    for mt in range(MT_LOC):
        xf = xf_tiles[mt]
        rs = st_p.tile([P, 1], F32, tag="rs", name=f"rs_{rep}_{mt}")
        nc.vector.tensor_tensor(out=rs, in0=r_tiles[mt], in1=a_s, op=ALU.mult)
        # xf <- xf * (r*a_s) + MAGIC ; z <- xf - MAGIC (exact int8 in bf16)
        nc.vector.tensor_scalar(
            out=xf[:], in0=xf[:], scalar1=rs, scalar2=MAGIC,
            op0=ALU.mult, op1=ALU.add,
        )
        z = zb_p.tile([P, K], BF16)
        nc.vector.tensor_scalar(
            out=z[:], in0=xf[:], scalar1=MAGIC, scalar2=None, op0=ALU.subtract
        )
        zt = ztp_p.tile([P, KT, P], BF16)
        nc.sync.dma_start_transpose(zt[:], z[:])
        dst = (zt_loc_a if mt == 0 else zt_loc_b)[:]
        nc.sync.dma_start(dst.rearrange("(p f) -> p f", p=P),
                          zt[:].rearrange("p a b -> p (a b)"))
        nc.gpsimd.collective_compute(
            "AllGather", ALU.bypass, replica_groups=rg,
            ins=[(zt_loc_a if mt == 0 else zt_loc_b)[:]],
            outs=[(zt_all_a if mt == 0 else zt_all_b)[:]],
        )

    # ---------- weight ternarization, JIT-chunked ---------------------------
    bt = bt_p.tile([P, KT, N_LOC], BF16)
    CH = 4
    for c in range(KT // CH):
        sl = w_res[:, c * CH : (c + 1) * CH, :]
        nc.vector.tensor_scalar(
            out=sl, in0=sl, scalar1=b_s[:, 0:1], scalar2=MAGIC,
            op0=ALU.mult, op1=ALU.add,
        )
        nc.vector.tensor_scalar(
            out=sl, in0=sl, scalar1=MAGIC, scalar2=1.0,
            op0=ALU.subtract, op1=ALU.min,
        )
        nc.vector.tensor_scalar(
            out=bt[:, c * CH : (c + 1) * CH, :], in0=sl, scalar1=-1.0,
            scalar2=None, op0=ALU.max,
        )

    return (bt, dq)


def emit_matmul(nc, zt_all_a, zt_all_b, out_ext, bt, dq,
                lhs_p, psum_p, out_p, rep):
    # half 0 consumes zt_all_a (each rank's first local m-tile: global m-tiles
    # 0,2,4,...), half 1 consumes zt_all_b (1,3,5,...), so the second
    # allgather overlaps the first half's matmuls. lhsT loads alternate DMA
    # queues to stay off the matmul's critical path.
    for half in range(2):
        base = (zt_all_a if half == 0 else zt_all_b)[:]
        psums = [
            psum_p.tile([P, N_LOC], F32, tag="ps", name=f"ps_{rep}_{half}_{i}")
            for i in range(R)
        ]
        for kt in range(KT):
            lhsT = lhs_p.tile([P, R * P], BF16, tag="lhsT",
                              name=f"lh_{rep}_{half}_{kt}")
            nc.sync.dma_start(
                lhsT[:].rearrange("p (r m) -> p r m", r=R),
                bass.AP(tensor=base.tensor, offset=kt * P,
                        ap=[[KT * P, P], [ZT_ELEMS, R], [1, P]]),
            )
            for r in range(R):
                nc.tensor.matmul(
                    psums[r][:],
                    lhsT[:, r * P : (r + 1) * P],
                    bt[:, kt, :],
                    start=(kt == 0),
                    stop=(kt == KT - 1),
                )
        for r in range(R):
            o_t = out_p.tile([P, N_LOC], F32)
            gm = 2 * r + half
            nc.vector.tensor_scalar_mul(
                out=o_t[:], in0=psums[r][:], scalar1=dq[:, 0:1]
            )
            nc.scalar.dma_start(
                out_ext[gm * P : (gm + 1) * P, :], o_t[:]
            )


_CACHE = {}


def _get_nc():
    if "nc" not in _CACHE:
        _CACHE["nc"] = build_kernel()
    return _CACHE["nc"]


def make_in_maps(x, weight, rms_weight):
    x = np.ascontiguousarray(np.asarray(x, dtype=np.float32)).reshape(M, K)
    weight = np.asarray(weight, dtype=np.float32)
    rms_weight = np.ascontiguousarray(np.asarray(rms_weight, dtype=np.float32))
    return [
        {
            "x_loc": np.ascontiguousarray(x[c * M_LOC : (c + 1) * M_LOC]),
            "w_loc": np.ascontiguousarray(weight[:, c * N_LOC : (c + 1) * N_LOC]),
            "rms_w": rms_weight,
        }
        for c in range(R)
    ]


def assemble_out(results):
    out = np.concatenate([results[c]["out_loc"] for c in range(R)], axis=1)
    return out.reshape(1, M, N)


def kernel(x, weight, rms_weight):
    nc = _get_nc()
    in_maps = make_in_maps(x, weight, rms_weight)
    res = run_bass_kernel_spmd(nc, in_maps, core_ids=list(range(R)))
    return assemble_out(res.results)
